# revision 1
# baseline (speedup 1.0000x reference)
"""Multi-head attention (B=2, T=2048, D=1024, R=16 heads, v=k) on 8 TRN2 cores.

Sharding: batch*heads across cores. Core c handles batch c//4, heads
[4*(c%4), 4*(c%4)+4). Each core computes its 4 heads' attention plus the
partial output projection; the host sums the 4 partials per batch.

Per-core dataflow (all matmul operands float32r = TF32-grade, full PE rate):
  qhT/khT [d, T] head-pair-major   <- wq/wk chunks (lhsT) x qT/kT chunks (rhs)
  vh      [T, d+ones]              <- kT chunks (lhsT) x wv chunks (rhs)
  S^T     [tk, tq]  row-tiled K=64 pairs (2 heads concurrently on PE halves)
  P^T = exp(S^T/8)  one ACT call per [128, 1024] pair tile
  PV      M=65 (64 d cols + ones col -> softmax denominator in row 64)
  scale   reciprocal(bcast(sums)) * PV  -> mergedT_h [64, T]
  outproj 4 accumulating K=64 matmuls -> out [T, 1024] partial
"""

import numpy as np

B, T, D = 2, 2048, 1024
R = 16
DH = 64
NCORES = 8
GROUPS = 4          # head groups (cores per batch)
HPG = 4             # heads per group/core
DG = HPG * DH       # 256 projected cols per core
NCHUNK = D // 128   # 8 contraction chunks
NTB = T // 128      # 16 t-blocks
NTQ = T // 512      # 4 tq tiles
VW = DH + 1         # 65: V columns + ones column

_CACHE = {}


def _build():
    import concourse.mybir as mybir
    import concourse.tile as tile
    from bass_rust import add_dep_helper
    from concourse import bacc

    FP32 = mybir.dt.float32
    FP32R = mybir.dt.float32r
    EXP = mybir.ActivationFunctionType.Exp

    nc = bacc.Bacc("TRN2", target_bir_lowering=False, debug=False)

    qT = nc.dram_tensor("qT", [D, T], FP32R, kind="ExternalInput")
    kT = nc.dram_tensor("kT", [D, T], FP32R, kind="ExternalInput")
    wq = nc.dram_tensor("wq", [D, DG], FP32R, kind="ExternalInput")
    wk = nc.dram_tensor("wk", [D, DG], FP32R, kind="ExternalInput")
    wv = nc.dram_tensor("wv", [D, DG], FP32R, kind="ExternalInput")
    wo = nc.dram_tensor("wo", [DG, D], FP32R, kind="ExternalInput")
    out = nc.dram_tensor("out", [T, D], FP32, kind="ExternalOutput")

    with tile.TileContext(nc) as tc:
        with (
            tc.tile_pool(name="weights", bufs=1) as wpool,
            tc.tile_pool(name="persist", bufs=1) as pers,
            tc.tile_pool(name="outstage", bufs=4) as ostage,
        ):
            wq_sb = wpool.tile([128, NCHUNK, DG], FP32R)
            wk_sb = wpool.tile([128, NCHUNK, DG], FP32R)
            wv_sb = wpool.tile([128, NCHUNK, DG], FP32R)
            nc.sync.dma_start(wq_sb[:], wq[:].rearrange("(c p) d -> p c d", p=128))
            nc.sync.dma_start(wk_sb[:], wk[:].rearrange("(c p) d -> p c d", p=128))
            nc.sync.dma_start(wv_sb[:], wv[:].rearrange("(c p) d -> p c d", p=128))
            wo_h = []
            for j in range(HPG):
                t_ = wpool.tile([DH, D], FP32R, tag=f"wo{j}", name=f"wo{j}")
                nc.sync.dma_start(t_[:], wo[j * DH : (j + 1) * DH, :])
                wo_h.append(t_)

            ones_f32 = pers.tile([128, DH], FP32, tag="ones_f32")
            nc.gpsimd.memset(ones_f32[:], 1.0)
            ones_sb = pers.tile([128, DH], FP32R, tag="ones")
            nc.vector.tensor_copy(ones_sb[:], ones_f32[:])
            warm_sb = pers.tile([128, 512], FP32, tag="warm_sb")
            nc.gpsimd.memset(warm_sb[:], 0.5)

            qhT = [pers.tile([128, T], FP32R, tag=f"qhT{p}", name=f"qhT{p}") for p in range(2)]
            khT = [pers.tile([128, T], FP32R, tag=f"khT{p}", name=f"khT{p}") for p in range(2)]
            vh = pers.tile([128, NTB, HPG, VW], FP32R, tag="vh")
            nc.vector.tensor_copy(
                vh[:, :, :, DH],
                ones_f32[:].rearrange("p (a b) -> p a b", a=NTB),
            )
            merged = [pers.tile([DH, T], FP32R, tag=f"mg{j}", name=f"mg{j}") for j in range(HPG)]

            # ---------- projection phases ----------
            with (
                tc.tile_pool(name="stream", bufs=8) as stream,
                tc.tile_pool(name="proj_ps", bufs=8, space="PSUM") as pps,
            ):
                # qhT: out[d, t] accumulated over D chunks
                qch = [stream.tile([128, T], FP32R, tag="stream", name=f"qch{c}") for c in range(NCHUNK)]
                for half in range(2):
                    hs = slice(half * (T // 2), (half + 1) * (T // 2))
                    for c in range(NCHUNK):
                        nc.sync.dma_start(qch[c][:, hs], qT[c * 128 : (c + 1) * 128, hs])
                acc = [pps.tile([128, 512], FP32, tag="pp", name=f"acc{i}") for i in range(8)]
                # PE warmup into acc[0]; first real start=True clears the bank
                for i in range(6):
                    nc.tensor.matmul(
                        acc[0][:], warm_sb[:, 0:128], warm_sb[:],
                        start=(i == 0), stop=True,
                    )
                for c in range(NCHUNK):
                    for p in range(2):
                        for tt in range(NTQ):
                            nc.tensor.matmul(
                                acc[p * NTQ + tt][:],
                                wq_sb[:, c, p * 128 : (p + 1) * 128],
                                qch[c][:, tt * 512 : (tt + 1) * 512],
                                start=(c == 0),
                                stop=(c == NCHUNK - 1),
                            )
                for p in range(2):
                    for tt in range(NTQ):
                        nc.vector.tensor_copy(
                            qhT[p][:, tt * 512 : (tt + 1) * 512],
                            acc[p * NTQ + tt][:],
                        )

                # kT chunks serve both vh and khT
                kch = [stream.tile([128, T], FP32R, tag="stream", name=f"kch{c}") for c in range(NCHUNK)]
                for half in range(2):
                    hs = slice(half * (T // 2), (half + 1) * (T // 2))
                    for c in range(NCHUNK):
                        nc.sync.dma_start(kch[c][:, hs], kT[c * 128 : (c + 1) * 128, hs])

                # vh: one accumulation group per PSUM bank (tb outer, chunk inner)
                for tb in range(NTB):
                    vacc = pps.tile([128, DG], FP32, tag="pp", name=f"vacc{tb}")
                    for c in range(NCHUNK):
                        nc.tensor.matmul(
                            vacc[:],
                            kch[c][:, tb * 128 : (tb + 1) * 128],
                            wv_sb[:, c, :],
                            start=(c == 0),
                            stop=(c == NCHUNK - 1),
                        )
                    nc.vector.tensor_copy(
                        vh[:, tb, :, 0:DH],
                        vacc[:].rearrange("p (j d) -> p j d", j=HPG),
                    )

                acc2 = [pps.tile([128, 512], FP32, tag="pp", name=f"kacc{i}") for i in range(8)]
                for c in range(NCHUNK):
                    for p in range(2):
                        for tt in range(NTQ):
                            nc.tensor.matmul(
                                acc2[p * NTQ + tt][:],
                                wk_sb[:, c, p * 128 : (p + 1) * 128],
                                kch[c][:, tt * 512 : (tt + 1) * 512],
                                start=(c == 0),
                                stop=(c == NCHUNK - 1),
                            )
                for p in range(2):
                    for tt in range(NTQ):
                        nc.vector.tensor_copy(
                            khT[p][:, tt * 512 : (tt + 1) * 512],
                            acc2[p * NTQ + tt][:],
                        )

            # ---------- attention + output projection ----------
            with (
                tc.tile_pool(name="pT", bufs=12) as ppool,
                tc.tile_pool(name="small", bufs=4) as small,
                tc.tile_pool(name="s_ps", bufs=2, space="PSUM") as sps,
                tc.tile_pool(name="pv_ps", bufs=2, space="PSUM") as pvps,
                tc.tile_pool(name="b_ps", bufs=1, space="PSUM") as bps,
                tc.tile_pool(name="o_ps", bufs=1, space="PSUM") as ops,
            ):
                pending_bcasts = []
                for tq in range(NTQ):
                    tqs = slice(tq * 512, (tq + 1) * 512)
                    for p in range(2):
                        pv = [
                            pvps.tile([128, 512], FP32, tag="pv", name=f"pv{h}")
                            for h in range(2)
                        ]
                        pTs = [None] * NTB

                        def do_pv(tk, after=None):
                            for h in range(2):
                                m = nc.tensor.matmul(
                                    pv[h][0:VW, :],
                                    vh[:, tk, p * 2 + h, :],
                                    pTs[tk][:, h * 512 : (h + 1) * 512],
                                    start=(tk == 0),
                                    stop=(tk == NTB - 1),
                                )
                                if after is not None:
                                    add_dep_helper(
                                        m.ins, after.ins,
                                        reason="PV ordered after next S pair",
                                    )

                        for tk in range(NTB):
                            s_ps = sps.tile([128, 1024], FP32, tag="s")
                            sB = None
                            for h in range(2):
                                lo, hi = h * 64, (h + 1) * 64
                                sB = nc.tensor.matmul(
                                    s_ps[:, h * 512 : (h + 1) * 512],
                                    khT[p][lo:hi, tk * 128 : (tk + 1) * 128],
                                    qhT[p][lo:hi, tqs],
                                    start=True,
                                    stop=True,
                                )
                            pTs[tk] = ppool.tile([128, 1024], FP32R, tag="pT", name=f"pT{tk}")
                            nc.scalar.activation(pTs[tk][:], s_ps[:], EXP, scale=0.125)
                            if tk == 2 and pending_bcasts:
                                for bi in pending_bcasts:
                                    add_dep_helper(
                                        bi.ins, sB.ins,
                                        reason="bcast after next segment start",
                                    )
                                pending_bcasts = []
                            if tk >= 1:
                                do_pv(tk - 1, after=sB)
                        do_pv(NTB - 1)

                        for h in range(2):
                            j = p * 2 + h
                            sums = small.tile([128, 512], FP32R, tag="sums")
                            nc.vector.tensor_copy(sums[64:65, :], pv[h][64:65, :])
                            bc = bps.tile([64, 512], FP32, tag="bc")
                            bm = nc.tensor.matmul(
                                bc[:],
                                ones_sb[64:65, :],
                                sums[64:65, :],
                                start=True,
                                stop=True,
                            )
                            pending_bcasts.append(bm)
                            rec = small.tile([64, 512], FP32, tag="rec")
                            nc.vector.reciprocal_approx_fast(rec[:], bc[:])
                            nc.vector.tensor_mul(
                                merged[j][:, tqs], pv[h][0:DH, :], rec[:]
                            )

                    # output projection, pipelined one tq tile behind
                    otq = tq - 1
                    if otq < 0:
                        continue
                    for tb in range(otq * 4, (otq + 1) * 4):
                        tbs = slice(tb * 128, (tb + 1) * 128)
                        for nt in range(2):
                            op = ops.tile([128, 512], FP32, tag="op")
                            for j in range(HPG):
                                nc.tensor.matmul(
                                    op[:],
                                    merged[j][:, tbs],
                                    wo_h[j][:, nt * 512 : (nt + 1) * 512],
                                    start=(j == 0),
                                    stop=(j == HPG - 1),
                                )
                            ob = ostage.tile([128, 512], FP32, tag="ob")
                            nc.vector.tensor_copy(ob[:], op[:])
                            nc.sync.dma_start(
                                out[tbs, nt * 512 : (nt + 1) * 512], ob[:]
                            )

                for tb in range((NTQ - 1) * 4, NTQ * 4):
                    tbs = slice(tb * 128, (tb + 1) * 128)
                    for nt in range(2):
                        op = ops.tile([128, 512], FP32, tag="op")
                        for j in range(HPG):
                            nc.tensor.matmul(
                                op[:],
                                merged[j][:, tbs],
                                wo_h[j][:, nt * 512 : (nt + 1) * 512],
                                start=(j == 0),
                                stop=(j == HPG - 1),
                            )
                        ob = ostage.tile([128, 512], FP32, tag="ob")
                        nc.vector.tensor_copy(ob[:], op[:])
                        nc.sync.dma_start(
                            out[tbs, nt * 512 : (nt + 1) * 512], ob[:]
                        )

    nc.compile()
    return nc


def _get_nc():
    if "nc" not in _CACHE:
        _CACHE["nc"] = _build()
    return _CACHE["nc"]


def kernel(q, k, q_map, k_map, v_map, output_map, trace=False):
    from concourse.bass_utils import run_bass_kernel_spmd

    q = np.asarray(q, dtype=np.float32)
    k = np.asarray(k, dtype=np.float32)
    q_map = np.asarray(q_map, dtype=np.float32)
    k_map = np.asarray(k_map, dtype=np.float32)
    v_map = np.asarray(v_map, dtype=np.float32)
    output_map = np.asarray(output_map, dtype=np.float32)

    nc = _get_nc()

    qTs = [np.ascontiguousarray(q[b].T) for b in range(B)]
    kTs = [np.ascontiguousarray(k[b].T) for b in range(B)]
    in_maps = []
    for c in range(NCORES):
        b, g = c // GROUPS, c % GROUPS
        cs = slice(g * DG, (g + 1) * DG)
        in_maps.append(
            {
                "qT": qTs[b],
                "kT": kTs[b],
                "wq": np.ascontiguousarray(q_map[:, cs]),
                "wk": np.ascontiguousarray(k_map[:, cs]),
                "wv": np.ascontiguousarray(v_map[:, cs]),
                "wo": np.ascontiguousarray(output_map[cs, :]),
            }
        )

    res = run_bass_kernel_spmd(nc, in_maps, list(range(NCORES)), trace=trace)
    if trace:
        _CACHE["last_exec_time_ns"] = res.exec_time_ns
        _CACHE["last_results"] = res

    outp = np.empty((B, T, D), dtype=np.float32)
    for b in range(B):
        acc = res.results[b * GROUPS]["out"].astype(np.float32)
        for g in range(1, GROUPS):
            acc = acc + res.results[b * GROUPS + g]["out"]
        outp[b] = acc
    return outp



# revision 2
# speedup vs baseline: 1.4717x; 1.4717x over previous
"""Multi-head attention (B=2, T=2048, D=1024, R=16 heads, v=k) on 8 TRN2 cores.

Sharding: batch*heads across cores. Core c handles batch c//4, heads
[4*(c%4), 4*(c%4)+4). Each core computes its 4 heads' attention plus the
partial output projection; the host sums the 4 partials per batch.

All matmul operands are bf16 so every moving stream fits the ~256B/cycle
XBUS word (fp32/fp32r K=128 movs run at half PE clock). Accumulation is
fp32 in PSUM; softmax exp runs on the scalar engine (ACT) at
(N+352)/1.2 ns per [128, N] tile, which is the attention-phase floor.

Per-core dataflow:
  qhT/khT [d, T] head-pair-major   <- wq/wk chunks (lhsT) x qT/kT chunks (rhs)
  vh      [T, d+ones]              <- kT chunks (lhsT) x wv chunks (rhs)
  S^T     [tk, tq]  row-tiled K=64 pairs (2 heads concurrent on PE halves)
  P^T = exp(S^T/8)  one ACT call per [128, 1024] pair tile -> bf16
  PV      M=65 (64 d cols + ones col -> softmax denominator in row 64)
  scale   reciprocal(bcast(sums)) * PV  -> mergedT_h [64, T] bf16
  outproj 4 accumulating K=64 matmuls -> out [T, 1024] partial fp32
"""

import numpy as np
import ml_dtypes

BF16 = ml_dtypes.bfloat16

B, T, D = 2, 2048, 1024
R = 16
DH = 64
NCORES = 8
GROUPS = 4          # head groups (cores per batch)
HPG = 4             # heads per group/core
DG = HPG * DH       # 256 projected cols per core
NCHUNK = D // 128   # 8 contraction chunks
NTB = T // 128      # 16 t-blocks
NTQ = T // 512      # 4 tq tiles
VW = DH + 1         # 65: V columns + ones column

_CACHE = {}


def _build():
    import concourse.mybir as mybir
    import concourse.tile as tile
    from bass_rust import add_dep_helper
    from concourse import bacc

    FP32 = mybir.dt.float32
    FP32R = mybir.dt.float32r
    BF = mybir.dt.bfloat16
    EXP = mybir.ActivationFunctionType.Exp

    nc = bacc.Bacc("TRN2", target_bir_lowering=False, debug=False)

    qT = nc.dram_tensor("qT", [D, T], BF, kind="ExternalInput")
    kT = nc.dram_tensor("kT", [D, T], BF, kind="ExternalInput")
    wq = nc.dram_tensor("wq", [D, DG], BF, kind="ExternalInput")
    wk = nc.dram_tensor("wk", [D, DG], BF, kind="ExternalInput")
    wv = nc.dram_tensor("wv", [D, DG], BF, kind="ExternalInput")
    wo = nc.dram_tensor("wo", [DG, D], BF, kind="ExternalInput")
    out = nc.dram_tensor("out", [T, D], FP32, kind="ExternalOutput")

    with tile.TileContext(nc) as tc:
        with (
            tc.tile_pool(name="weights", bufs=1) as wpool,
            tc.tile_pool(name="persist", bufs=1) as pers,
            tc.tile_pool(name="outstage", bufs=8) as ostage,
        ):
            wq_sb = wpool.tile([128, NCHUNK, DG], BF)
            wk_sb = wpool.tile([128, NCHUNK, DG], BF)
            wv_sb = wpool.tile([128, NCHUNK, DG], BF)
            wo_h = [
                wpool.tile([DH, D], BF, tag=f"wo{j}", name=f"wo{j}")
                for j in range(HPG)
            ]

            ones_f32 = pers.tile([128, DH], FP32, tag="ones_f32")
            nc.gpsimd.memset(ones_f32[:], 1.0)
            ones_sb = pers.tile([128, DH], FP32R, tag="ones")
            nc.vector.tensor_copy(ones_sb[:], ones_f32[:])
            warm_sb = pers.tile([128, 512], BF, tag="warm_sb")
            nc.gpsimd.memset(warm_sb[:], 0.5)

            qhT = [pers.tile([128, T], BF, tag=f"qhT{p}", name=f"qhT{p}") for p in range(2)]
            khT = [pers.tile([128, T], BF, tag=f"khT{p}", name=f"khT{p}") for p in range(2)]
            vh = pers.tile([128, NTB, HPG, VW], BF, tag="vh")
            nc.vector.tensor_copy(
                vh[:, :, :, DH],
                ones_f32[:].rearrange("p (a b) -> p a b", a=NTB),
            )
            merged = [pers.tile([DH, T], BF, tag=f"mg{j}", name=f"mg{j}") for j in range(HPG)]

            # ---------- projection phases ----------
            with (
                tc.tile_pool(name="stream", bufs=8) as stream,
                tc.tile_pool(name="proj_ps", bufs=8, space="PSUM") as pps,
            ):
                # DMA order matters: wq first, then qT chunks (c-major so
                # chunk 0 lands fast), then wv + kT chunks, then wk, wo.
                nc.sync.dma_start(wq_sb[:], wq[:].rearrange("(c p) d -> p c d", p=128))
                qch = [stream.tile([128, T], BF, tag="stream", name=f"qch{c}") for c in range(NCHUNK)]
                for c in range(NCHUNK):
                    nc.sync.dma_start(qch[c][:], qT[c * 128 : (c + 1) * 128, :])
                nc.sync.dma_start(wv_sb[:], wv[:].rearrange("(c p) d -> p c d", p=128))
                kch = [stream.tile([128, T], BF, tag="stream", name=f"kch{c}") for c in range(NCHUNK)]
                for c in range(NCHUNK):
                    nc.sync.dma_start(kch[c][:], kT[c * 128 : (c + 1) * 128, :])
                nc.sync.dma_start(wk_sb[:], wk[:].rearrange("(c p) d -> p c d", p=128))
                for j in range(HPG):
                    nc.sync.dma_start(wo_h[j][:], wo[j * DH : (j + 1) * DH, :])

                # qhT: out[d, t] accumulated over D chunks
                acc = [pps.tile([128, 512], FP32, tag="pp", name=f"acc{i}") for i in range(8)]
                # PE warmup into acc[0]; first real start=True clears the bank
                for i in range(6):
                    nc.tensor.matmul(
                        acc[0][:], warm_sb[:, 0:128], warm_sb[:],
                        start=(i == 0), stop=True,
                    )
                for c in range(NCHUNK):
                    for p in range(2):
                        for tt in range(NTQ):
                            nc.tensor.matmul(
                                acc[p * NTQ + tt][:],
                                wq_sb[:, c, p * 128 : (p + 1) * 128],
                                qch[c][:, tt * 512 : (tt + 1) * 512],
                                start=(c == 0),
                                stop=(c == NCHUNK - 1),
                            )
                for p in range(2):
                    for tt in range(NTQ):
                        nc.vector.tensor_copy(
                            qhT[p][:, tt * 512 : (tt + 1) * 512],
                            acc[p * NTQ + tt][:],
                        )

                # vh: one accumulation group per PSUM bank (tb outer, chunk inner)
                for tb in range(NTB):
                    vacc = pps.tile([128, DG], FP32, tag="pp", name=f"vacc{tb}")
                    for c in range(NCHUNK):
                        nc.tensor.matmul(
                            vacc[:],
                            kch[c][:, tb * 128 : (tb + 1) * 128],
                            wv_sb[:, c, :],
                            start=(c == 0),
                            stop=(c == NCHUNK - 1),
                        )
                    nc.vector.tensor_copy(
                        vh[:, tb, :, 0:DH],
                        vacc[:].rearrange("p (j d) -> p j d", j=HPG),
                    )

                acc2 = [pps.tile([128, 512], FP32, tag="pp", name=f"kacc{i}") for i in range(8)]
                for c in range(NCHUNK):
                    for p in range(2):
                        for tt in range(NTQ):
                            nc.tensor.matmul(
                                acc2[p * NTQ + tt][:],
                                wk_sb[:, c, p * 128 : (p + 1) * 128],
                                kch[c][:, tt * 512 : (tt + 1) * 512],
                                start=(c == 0),
                                stop=(c == NCHUNK - 1),
                            )
                for p in range(2):
                    for tt in range(NTQ):
                        nc.vector.tensor_copy(
                            khT[p][:, tt * 512 : (tt + 1) * 512],
                            acc2[p * NTQ + tt][:],
                        )

            # ---------- attention + output projection ----------
            with (
                tc.tile_pool(name="pT", bufs=12) as ppool,
                tc.tile_pool(name="small", bufs=4) as small,
                tc.tile_pool(name="s_ps", bufs=2, space="PSUM") as sps,
                tc.tile_pool(name="pv_ps", bufs=2, space="PSUM") as pvps,
                tc.tile_pool(name="b_ps", bufs=1, space="PSUM") as bps,
                tc.tile_pool(name="o_ps", bufs=1, space="PSUM") as ops,
            ):
                pending_bcasts = []
                for tq in range(NTQ):
                    tqs = slice(tq * 512, (tq + 1) * 512)
                    for p in range(2):
                        pv = [
                            pvps.tile([128, 512], FP32, tag="pv", name=f"pv{h}")
                            for h in range(2)
                        ]
                        pTs = [None] * NTB

                        def do_pv(tk, after=None):
                            for h in range(2):
                                m = nc.tensor.matmul(
                                    pv[h][0:VW, :],
                                    vh[:, tk, p * 2 + h, :],
                                    pTs[tk][:, h * 512 : (h + 1) * 512],
                                    start=(tk == 0),
                                    stop=(tk == NTB - 1),
                                )
                                if after is not None:
                                    add_dep_helper(
                                        m.ins, after.ins,
                                        reason="PV ordered after next S pair",
                                    )

                        for tk in range(NTB):
                            s_ps = sps.tile([128, 1024], FP32, tag="s")
                            sB = None
                            for h in range(2):
                                lo, hi = h * 64, (h + 1) * 64
                                sB = nc.tensor.matmul(
                                    s_ps[:, h * 512 : (h + 1) * 512],
                                    khT[p][lo:hi, tk * 128 : (tk + 1) * 128],
                                    qhT[p][lo:hi, tqs],
                                    start=True,
                                    stop=True,
                                )
                            pTs[tk] = ppool.tile([128, 1024], BF, tag="pT", name=f"pT{tk}")
                            nc.scalar.activation(pTs[tk][:], s_ps[:], EXP, scale=0.125)
                            if tk == 2 and pending_bcasts:
                                for bi in pending_bcasts:
                                    add_dep_helper(
                                        bi.ins, sB.ins,
                                        reason="bcast after next segment start",
                                    )
                                pending_bcasts = []
                            if tk >= 1:
                                do_pv(tk - 1, after=sB)
                        do_pv(NTB - 1)

                        for h in range(2):
                            j = p * 2 + h
                            sums = small.tile([128, 512], FP32R, tag="sums")
                            nc.vector.tensor_copy(sums[64:65, :], pv[h][64:65, :])
                            bc = bps.tile([64, 512], FP32, tag="bc")
                            bm = nc.tensor.matmul(
                                bc[:],
                                ones_sb[64:65, :],
                                sums[64:65, :],
                                start=True,
                                stop=True,
                            )
                            pending_bcasts.append(bm)
                            rec = small.tile([64, 512], FP32, tag="rec")
                            nc.vector.reciprocal_approx_fast(rec[:], bc[:])
                            nc.vector.tensor_mul(
                                merged[j][:, tqs], pv[h][0:DH, :], rec[:]
                            )

                    # output projection, pipelined one tq tile behind
                    otq = tq - 1
                    if otq < 0:
                        continue
                    for tb in range(otq * 4, (otq + 1) * 4):
                        tbs = slice(tb * 128, (tb + 1) * 128)
                        for nt in range(2):
                            op = ops.tile([128, 512], FP32, tag="op")
                            for j in range(HPG):
                                nc.tensor.matmul(
                                    op[:],
                                    merged[j][:, tbs],
                                    wo_h[j][:, nt * 512 : (nt + 1) * 512],
                                    start=(j == 0),
                                    stop=(j == HPG - 1),
                                )
                            ob = ostage.tile([128, 512], FP32, tag="ob")
                            nc.vector.tensor_copy(ob[:], op[:])
                            nc.sync.dma_start(
                                out[tbs, nt * 512 : (nt + 1) * 512], ob[:]
                            )

                for tb in range((NTQ - 1) * 4, NTQ * 4):
                    tbs = slice(tb * 128, (tb + 1) * 128)
                    for nt in range(2):
                        op = ops.tile([128, 512], FP32, tag="op")
                        for j in range(HPG):
                            nc.tensor.matmul(
                                op[:],
                                merged[j][:, tbs],
                                wo_h[j][:, nt * 512 : (nt + 1) * 512],
                                start=(j == 0),
                                stop=(j == HPG - 1),
                            )
                        ob = ostage.tile([128, 512], FP32, tag="ob")
                        nc.vector.tensor_copy(ob[:], op[:])
                        nc.sync.dma_start(
                            out[tbs, nt * 512 : (nt + 1) * 512], ob[:]
                        )

    nc.compile()
    return nc


def _get_nc():
    if "nc" not in _CACHE:
        _CACHE["nc"] = _build()
    return _CACHE["nc"]


def kernel(q, k, q_map, k_map, v_map, output_map, trace=False):
    from concourse.bass_utils import run_bass_kernel_spmd

    q = np.asarray(q, dtype=np.float32)
    k = np.asarray(k, dtype=np.float32)
    q_map = np.asarray(q_map, dtype=np.float32)
    k_map = np.asarray(k_map, dtype=np.float32)
    v_map = np.asarray(v_map, dtype=np.float32)
    output_map = np.asarray(output_map, dtype=np.float32)

    nc = _get_nc()

    qTs = [np.ascontiguousarray(q[b].T).astype(BF16) for b in range(B)]
    kTs = [np.ascontiguousarray(k[b].T).astype(BF16) for b in range(B)]
    in_maps = []
    for c in range(NCORES):
        b, g = c // GROUPS, c % GROUPS
        cs = slice(g * DG, (g + 1) * DG)
        in_maps.append(
            {
                "qT": qTs[b],
                "kT": kTs[b],
                "wq": np.ascontiguousarray(q_map[:, cs]).astype(BF16),
                "wk": np.ascontiguousarray(k_map[:, cs]).astype(BF16),
                "wv": np.ascontiguousarray(v_map[:, cs]).astype(BF16),
                "wo": np.ascontiguousarray(output_map[cs, :]).astype(BF16),
            }
        )

    res = run_bass_kernel_spmd(nc, in_maps, list(range(NCORES)), trace=trace)
    if trace:
        _CACHE["last_exec_time_ns"] = res.exec_time_ns
        _CACHE["last_results"] = res

    outp = np.empty((B, T, D), dtype=np.float32)
    for b in range(B):
        acc = res.results[b * GROUPS]["out"].astype(np.float32)
        for g in range(1, GROUPS):
            acc = acc + res.results[b * GROUPS + g]["out"]
        outp[b] = acc
    return outp


# revision 6
# speedup vs baseline: 1.5976x; 1.0856x over previous
"""Multi-head attention (B=2, T=2048, D=1024, R=16 heads, v=k) on 8 TRN2 cores.

Sharding: batch*heads across cores. Core c handles batch c//4, heads
[4*(c%4), 4*(c%4)+4). Each core computes its 4 heads' attention plus the
partial output projection; the host sums the 4 partials per batch.

All matmul operands are bf16 so every moving stream fits the ~256B/cycle
XBUS word (fp32/fp32r K=128 movs run at half PE clock). Accumulation is
fp32 in PSUM; softmax exp runs on the scalar engine (ACT) at
(N+352)/1.2 ns per [128, N] tile, which is the attention-phase floor.

Per-core dataflow:
  qhT/khT [d, T] head-pair-major   <- wq/wk chunks (lhsT) x qT/kT chunks (rhs)
  vh      [T, d+ones]              <- kT chunks (lhsT) x wv chunks (rhs)
  S^T     [tk, tq]  row-tiled K=64 pairs (2 heads concurrent on PE halves)
  P^T = exp(S^T/8)  one ACT call per [128, 1024] pair tile -> bf16
  PV      M=65 (64 d cols + ones col -> softmax denominator in row 64)
  scale   reciprocal(bcast(sums)) * PV  -> mergedT_h [64, T] bf16
  outproj 4 accumulating K=64 matmuls -> out [T, 1024] partial fp32
"""

import numpy as np
import ml_dtypes

BF16 = ml_dtypes.bfloat16

B, T, D = 2, 2048, 1024
R = 16
DH = 64
NCORES = 8
GROUPS = 4          # head groups (cores per batch)
HPG = 4             # heads per group/core
DG = HPG * DH       # 256 projected cols per core
NCHUNK = D // 128   # 8 contraction chunks
NTB = T // 128      # 16 t-blocks
NTQ = T // 512      # 4 tq tiles
VW = DH + 1         # 65: V columns + ones column

_CACHE = {}


def _build():
    import concourse.mybir as mybir
    import concourse.tile as tile
    from bass_rust import add_dep_helper
    from concourse import bacc

    FP32 = mybir.dt.float32
    FP32R = mybir.dt.float32r
    BF = mybir.dt.bfloat16
    EXP = mybir.ActivationFunctionType.Exp

    nc = bacc.Bacc("TRN2", target_bir_lowering=False, debug=False)

    qT = nc.dram_tensor("qT", [D, T], BF, kind="ExternalInput")
    kT = nc.dram_tensor("kT", [D, T], BF, kind="ExternalInput")
    wq = nc.dram_tensor("wq", [D, DG], BF, kind="ExternalInput")
    wk = nc.dram_tensor("wk", [D, DG], BF, kind="ExternalInput")
    wv = nc.dram_tensor("wv", [D, DG], BF, kind="ExternalInput")
    wo = nc.dram_tensor("wo", [DG, D], BF, kind="ExternalInput")
    out = nc.dram_tensor("out", [T, D], FP32, kind="ExternalOutput")

    with tile.TileContext(nc) as tc:
        with (
            tc.tile_pool(name="weights", bufs=1) as wpool,
            tc.tile_pool(name="persist", bufs=1) as pers,
            tc.tile_pool(name="outstage", bufs=8) as ostage,
        ):
            wq_sb = wpool.tile([128, NCHUNK, DG], BF)
            wk_sb = wpool.tile([128, NCHUNK, DG], BF)
            wv_sb = wpool.tile([128, NCHUNK, DG], BF)
            wo_h = [
                wpool.tile([DH, D], BF, tag=f"wo{j}", name=f"wo{j}")
                for j in range(HPG)
            ]

            ones_f32 = pers.tile([128, DH], FP32, tag="ones_f32")
            nc.gpsimd.memset(ones_f32[:], 1.0)
            ones_sb = pers.tile([128, DH], FP32R, tag="ones")
            nc.vector.tensor_copy(ones_sb[:], ones_f32[:])
            warm_sb = pers.tile([128, 512], BF, tag="warm_sb")
            nc.gpsimd.memset(warm_sb[:], 0.5)

            qhT = [pers.tile([128, T], BF, tag=f"qhT{p}", name=f"qhT{p}") for p in range(2)]
            khT = [pers.tile([128, T], BF, tag=f"khT{p}", name=f"khT{p}") for p in range(2)]
            vh = pers.tile([128, NTB, HPG, VW], BF, tag="vh")
            nc.vector.tensor_copy(
                vh[:, :, :, DH],
                ones_f32[:].rearrange("p (a b) -> p a b", a=NTB),
            )
            merged = [pers.tile([DH, T], BF, tag=f"mg{j}", name=f"mg{j}") for j in range(HPG)]

            # ---------- projection phases ----------
            with (
                tc.tile_pool(name="stream", bufs=8) as stream,
                tc.tile_pool(name="proj_ps", bufs=8, space="PSUM") as pps,
            ):
                # DMA order matters: wq first, then qT chunks (c-major so
                # chunk 0 lands fast), then wv + kT chunks, then wk, wo.
                nc.sync.dma_start(wq_sb[:], wq[:].rearrange("(c p) d -> p c d", p=128))
                qch = [stream.tile([128, T], BF, tag="stream", name=f"qch{c}") for c in range(NCHUNK)]
                for c in range(NCHUNK):
                    nc.sync.dma_start(qch[c][:], qT[c * 128 : (c + 1) * 128, :])
                nc.sync.dma_start(wv_sb[:], wv[:].rearrange("(c p) d -> p c d", p=128))
                kch = [stream.tile([128, T], BF, tag="stream", name=f"kch{c}") for c in range(NCHUNK)]
                for c in range(NCHUNK):
                    nc.sync.dma_start(kch[c][:], kT[c * 128 : (c + 1) * 128, :])
                nc.sync.dma_start(wk_sb[:], wk[:].rearrange("(c p) d -> p c d", p=128))
                for j in range(HPG):
                    nc.sync.dma_start(wo_h[j][:], wo[j * DH : (j + 1) * DH, :])

                # qhT: out[d, t] accumulated over D chunks
                acc = [pps.tile([128, 512], FP32, tag="pp", name=f"acc{i}") for i in range(8)]
                # PE warmup into acc[0]; first real start=True clears the bank
                for i in range(6):
                    nc.tensor.matmul(
                        acc[0][:], warm_sb[:, 0:128], warm_sb[:],
                        start=(i == 0), stop=True,
                    )
                for c in range(NCHUNK):
                    for p in range(2):
                        for tt in range(NTQ):
                            nc.tensor.matmul(
                                acc[p * NTQ + tt][:],
                                wq_sb[:, c, p * 128 : (p + 1) * 128],
                                qch[c][:, tt * 512 : (tt + 1) * 512],
                                start=(c == 0),
                                stop=(c == NCHUNK - 1),
                            )
                for p in range(2):
                    for tt in range(NTQ):
                        nc.vector.tensor_copy(
                            qhT[p][:, tt * 512 : (tt + 1) * 512],
                            acc[p * NTQ + tt][:],
                        )

                # vh: one accumulation group per PSUM bank (tb outer, chunk inner)
                for tb in range(NTB):
                    vacc = pps.tile([128, DG], FP32, tag="pp", name=f"vacc{tb}")
                    for c in range(NCHUNK):
                        nc.tensor.matmul(
                            vacc[:],
                            kch[c][:, tb * 128 : (tb + 1) * 128],
                            wv_sb[:, c, :],
                            start=(c == 0),
                            stop=(c == NCHUNK - 1),
                        )
                    nc.vector.tensor_copy(
                        vh[:, tb, :, 0:DH],
                        vacc[:].rearrange("p (j d) -> p j d", j=HPG),
                    )

                acc2 = [pps.tile([128, 512], FP32, tag="pp", name=f"kacc{i}") for i in range(8)]
                for c in range(NCHUNK):
                    for p in range(2):
                        for tt in range(NTQ):
                            nc.tensor.matmul(
                                acc2[p * NTQ + tt][:],
                                wk_sb[:, c, p * 128 : (p + 1) * 128],
                                kch[c][:, tt * 512 : (tt + 1) * 512],
                                start=(c == 0),
                                stop=(c == NCHUNK - 1),
                            )
                for p in range(2):
                    for tt in range(NTQ):
                        nc.vector.tensor_copy(
                            khT[p][:, tt * 512 : (tt + 1) * 512],
                            acc2[p * NTQ + tt][:],
                        )

            # ---------- attention + output projection ----------
            with (
                tc.tile_pool(name="pT", bufs=12) as ppool,
                tc.tile_pool(name="small", bufs=4) as small,
                tc.tile_pool(name="s_ps", bufs=2, space="PSUM") as sps,
                tc.tile_pool(name="pv_ps", bufs=2, space="PSUM") as pvps,
                tc.tile_pool(name="b_ps", bufs=1, space="PSUM") as bps,
                tc.tile_pool(name="o_ps", bufs=1, space="PSUM") as ops,
            ):
                def make_norm(pv, tqs, p):
                    # softmax normalize for the finished segment: denominator
                    # (pv row 64) -> PE broadcast -> reciprocal -> scale.
                    # Emitted early in the NEXT segment so its matmuls never
                    # sit ahead of that segment's S pairs in the PE queue.
                    def norm():
                        for h in range(2):
                            j = p * 2 + h
                            sums = small.tile([128, 512], FP32R, tag="sums")
                            nc.vector.tensor_copy(sums[64:65, :], pv[h][64:65, :])
                            bc = bps.tile([64, 512], FP32, tag="bc")
                            nc.tensor.matmul(
                                bc[:],
                                ones_sb[64:65, :],
                                sums[64:65, :],
                                start=True,
                                stop=True,
                            )
                            rec = small.tile([64, 512], FP32, tag="rec")
                            nc.vector.reciprocal_approx_fast(rec[:], bc[:])
                            nc.vector.tensor_mul(
                                merged[j][:, tqs], pv[h][0:DH, :], rec[:]
                            )
                    return norm

                def make_outproj(otq, pool, tagname):
                    # one closure per 128x512 out tile; spread across tk slots
                    tiles = []
                    for tb in range(otq * 4, (otq + 1) * 4):
                        tbs = slice(tb * 128, (tb + 1) * 128)
                        for nt in range(2):
                            def emit(tbs=tbs, nt=nt):
                                op = pool.tile([128, 512], FP32, tag=tagname, name="op")
                                for j in range(HPG):
                                    nc.tensor.matmul(
                                        op[:],
                                        merged[j][:, tbs],
                                        wo_h[j][:, nt * 512 : (nt + 1) * 512],
                                        start=(j == 0),
                                        stop=(j == HPG - 1),
                                    )
                                ob = ostage.tile([128, 512], FP32, tag="ob")
                                nc.vector.tensor_copy(ob[:], op[:])
                                nc.sync.dma_start(
                                    out[tbs, nt * 512 : (nt + 1) * 512], ob[:]
                                )
                            tiles.append(emit)
                    return tiles

                pending_norm = None
                pending_outproj = []
                for tq in range(NTQ):
                    tqs = slice(tq * 512, (tq + 1) * 512)
                    for p in range(2):
                        pv = [
                            pvps.tile([128, 512], FP32, tag="pv", name=f"pv{h}")
                            for h in range(2)
                        ]
                        pTs = [None] * NTB

                        def do_pv(tk, after=None):
                            for h in range(2):
                                m = nc.tensor.matmul(
                                    pv[h][0:VW, :],
                                    vh[:, tk, p * 2 + h, :],
                                    pTs[tk][:, h * 512 : (h + 1) * 512],
                                    start=(tk == 0),
                                    stop=(tk == NTB - 1),
                                )
                                if after is not None:
                                    add_dep_helper(
                                        m.ins, after.ins,
                                        reason="PV ordered after next S pair",
                                    )

                        for tk in range(NTB):
                            s_ps = sps.tile([128, 1024], FP32, tag="s")
                            sB = None
                            for h in range(2):
                                lo, hi = h * 64, (h + 1) * 64
                                sB = nc.tensor.matmul(
                                    s_ps[:, h * 512 : (h + 1) * 512],
                                    khT[p][lo:hi, tk * 128 : (tk + 1) * 128],
                                    qhT[p][lo:hi, tqs],
                                    start=True,
                                    stop=True,
                                )
                            pTs[tk] = ppool.tile([128, 1024], BF, tag="pT", name=f"pT{tk}")
                            nc.scalar.activation(pTs[tk][:], s_ps[:], EXP, scale=0.125)
                            if tk == 1 and pending_norm is not None:
                                pending_norm()
                                pending_norm = None
                            if tk in (3, 5, 7, 9) and pending_outproj:
                                pending_outproj.pop(0)()
                            if tk >= 1:
                                do_pv(tk - 1, after=sB)
                        do_pv(NTB - 1)
                        pending_norm = make_norm(pv, tqs, p)

                    if tq >= 1:
                        pending_outproj.extend(make_outproj(tq - 1, ops, "op"))

                # drain leftover interleaved outproj tiles, then normalize
                # the final segment
                for emit in pending_outproj:
                    emit()
                pending_norm()

            # tail: last tq's output projection on freed PSUM banks
            with tc.tile_pool(name="tail_ps", bufs=4, space="PSUM") as tps:
                for tb in range((NTQ - 1) * 4, NTQ * 4):
                    tbs = slice(tb * 128, (tb + 1) * 128)
                    for nt in range(2):
                        op = tps.tile([128, 512], FP32, tag="top")
                        for j in range(HPG):
                            nc.tensor.matmul(
                                op[:],
                                merged[j][:, tbs],
                                wo_h[j][:, nt * 512 : (nt + 1) * 512],
                                start=(j == 0),
                                stop=(j == HPG - 1),
                            )
                        ob = ostage.tile([128, 512], FP32, tag="ob")
                        nc.vector.tensor_copy(ob[:], op[:])
                        nc.sync.dma_start(
                            out[tbs, nt * 512 : (nt + 1) * 512], ob[:]
                        )

    nc.compile()
    return nc


def _get_nc():
    if "nc" not in _CACHE:
        _CACHE["nc"] = _build()
    return _CACHE["nc"]


def kernel(q, k, q_map, k_map, v_map, output_map, trace=False):
    from concourse.bass_utils import run_bass_kernel_spmd

    q = np.asarray(q, dtype=np.float32)
    k = np.asarray(k, dtype=np.float32)
    q_map = np.asarray(q_map, dtype=np.float32)
    k_map = np.asarray(k_map, dtype=np.float32)
    v_map = np.asarray(v_map, dtype=np.float32)
    output_map = np.asarray(output_map, dtype=np.float32)

    nc = _get_nc()

    qTs = [np.ascontiguousarray(q[b].T).astype(BF16) for b in range(B)]
    kTs = [np.ascontiguousarray(k[b].T).astype(BF16) for b in range(B)]
    in_maps = []
    for c in range(NCORES):
        b, g = c // GROUPS, c % GROUPS
        cs = slice(g * DG, (g + 1) * DG)
        in_maps.append(
            {
                "qT": qTs[b],
                "kT": kTs[b],
                "wq": np.ascontiguousarray(q_map[:, cs]).astype(BF16),
                "wk": np.ascontiguousarray(k_map[:, cs]).astype(BF16),
                "wv": np.ascontiguousarray(v_map[:, cs]).astype(BF16),
                "wo": np.ascontiguousarray(output_map[cs, :]).astype(BF16),
            }
        )

    res = run_bass_kernel_spmd(nc, in_maps, list(range(NCORES)), trace=trace)
    if trace:
        _CACHE["last_exec_time_ns"] = res.exec_time_ns
        _CACHE["last_results"] = res

    outp = np.empty((B, T, D), dtype=np.float32)
    for b in range(B):
        acc = res.results[b * GROUPS]["out"].astype(np.float32)
        for g in range(1, GROUPS):
            acc = acc + res.results[b * GROUPS + g]["out"]
        outp[b] = acc
    return outp
